# revision 37
# baseline (speedup 1.0000x reference)
"""Multi-head attention (b=2, n=2048, d_model=1024, H=16, d_k=d_v=64) on 8
Trainium2 NeuronCores.

Sharding: 8 cores = 2 (batch) x 4 (head groups of 4 heads).  Each core
computes, for its batch ib and head group g:

    q/k projections in transposed form  qT = Wq_g @ x^T   [256, 2048]
    v projection in natural form        V  = x @ Wv_g^T   [2048, 256]
    per head: S^T = K_h Q_h^T (k on partitions), A^T = exp(S^T/8),
              O^T|Z = [V_h|1]^T A^T  (PE row 64 gives softmax denom Z)
    normalize O^T by 1/Z, out-projection Y^T = Wo_g @ O_cat^T  [1024, 2048]

Host sums the 4 per-group partial Y^T per batch and adds bo.

All matmuls in bf16 (fp32 PSUM).  The attention inner loop is software-
pipelined with a skew of 2 so the PE never waits on the exp: per
iteration (kt, hp) the PE runs S(it) then AV(it-2) while exp(it) runs
concurrently, SPLIT between the Scalar engine (true Exp on one query
half) and the DVE (Schraudolph bit-trick exp on the other: bf16 bits =
round(score*A + B) as one scalar_tensor_tensor into int16, bitcast to
bf16; softmax normalization cancels most of the sawtooth).  The halves
alternate with kt so every query mixes exact and approximate slices.
Each engine gets its OWN S-score PSUM tile and its own at-tile ring --
sharing either re-serializes the engines through Tile's semaphore
proxying.  Softmax denominators: PSUM row 64 -> SBUF (scalar copy) ->
reciprocal (DVE) -> DRAM bounce -> partition-broadcast DMA -> O
multiply (DVE), all off the PE critical path.
"""

import math
import numpy as np
import ml_dtypes
from contextlib import ExitStack

import concourse.bass as bass
import concourse.mybir as mybir
import concourse.tile as tile
from concourse import bacc
from concourse.bass_utils import run_bass_kernel_spmd

F32 = mybir.dt.float32
BF16 = mybir.dt.bfloat16
I16 = mybir.dt.int16
EXP = mybir.ActivationFunctionType.Exp
COPY = mybir.ActivationFunctionType.Copy
IDENT = mybir.ActivationFunctionType.Identity
ADD = mybir.AluOpType.add
MULT = mybir.AluOpType.mult

NP_BF16 = ml_dtypes.bfloat16

D_MODEL = 1024
H = 16
DK = 64
B = 2
N = 2048           # nq = nk
G = 4              # head groups (cores per batch)
HG = H // G        # heads per group = 4
DG = HG * DK       # 256 group dims
KT = 8             # D_MODEL / 128 contraction tiles
NKT = N // 128     # 16 k-tiles in attention
QC = 1024          # attention q-chunk
NCH = N // QC      # 2 chunks
P = 128

# Schraudolph-style exp in bf16 bit space, with the 1/8 softmax scale
# folded in: bf16_bits(e^(x/8)) ~= x * SCH_A + SCH_B
_LN2 = math.log(2.0)
SCH_A = 0.125 * 128.0 / _LN2
SCH_B = 16256.0 - 128.0 * ((2.0 * _LN2 - 1.0 - _LN2 / 2.0) / _LN2)

_PROGRAM = None


def _build_program():
    nc = bacc.Bacc("TRN2", target_bir_lowering=False, debug=False, num_devices=8)

    xqT = nc.dram_tensor("xqT", [4, P, KT, 512], BF16, kind="ExternalInput").ap()
    xkT = nc.dram_tensor("xkT", [4, P, KT, 512], BF16, kind="ExternalInput").ap()
    xvT = nc.dram_tensor("xvT", [NKT, P, KT, P], BF16, kind="ExternalInput").ap()
    wqT = nc.dram_tensor("wqT", [P, KT, DG], BF16, kind="ExternalInput").ap()
    wkT = nc.dram_tensor("wkT", [P, KT, DG], BF16, kind="ExternalInput").ap()
    wvT = nc.dram_tensor("wvT", [P, KT, DG], BF16, kind="ExternalInput").ap()
    woT = nc.dram_tensor("woT", [P, 2, D_MODEL], BF16, kind="ExternalInput").ap()
    bq_d = nc.dram_tensor("bq_s", [DG], F32, kind="ExternalInput").ap()
    bk_d = nc.dram_tensor("bk_s", [DG], F32, kind="ExternalInput").ap()
    bv_d = nc.dram_tensor("bv_s", [DG], BF16, kind="ExternalInput").ap()
    ones_d = nc.dram_tensor("ones_c", [P], BF16, kind="ExternalInput").ap()
    yT_d = nc.dram_tensor("yT", [D_MODEL, N], F32, kind="ExternalOutput").ap()
    # dram staging for softmax reciprocal rows (partition-broadcast DMA needs
    # a DRAM source; internal DRAM tiles fail under axon PJRT, so this is an
    # ExternalOutput buffer)
    rz_st = nc.dram_tensor("rz_st", [2 * NCH * 2, QC], F32,
                           kind="ExternalOutput").ap()

    bq_v = bq_d.rearrange("(j p) -> p j", p=P)        # [128, 2]
    bk_v = bk_d.rearrange("(j p) -> p j", p=P)

    with tile.TileContext(nc) as tc:
        with ExitStack() as ctx:
            const = ctx.enter_context(tc.tile_pool(name="const", bufs=1))
            xin = ctx.enter_context(tc.tile_pool(name="xin", bufs=2))
            xvp = ctx.enter_context(tc.tile_pool(name="xvp", bufs=3))
            work = ctx.enter_context(tc.tile_pool(name="work", bufs=2))
            atp = ctx.enter_context(tc.tile_pool(name="atp", bufs=6))
            smal = ctx.enter_context(tc.tile_pool(name="smal", bufs=2))
            pp = ctx.enter_context(tc.tile_pool(name="pp", bufs=2, space="PSUM"))

            # ---- constants ----
            wk_sb = const.tile([P, KT, DG], BF16, tag="wk", name="wk_sb")
            wv_sb = const.tile([P, KT, DG], BF16, tag="wv", name="wv_sb")
            wq_sb = const.tile([P, KT, DG], BF16, tag="wq", name="wq_sb")
            wo_sb = const.tile([P, 2, D_MODEL], BF16, tag="wo", name="wo_sb")
            nc.scalar.dma_start(wq_sb[:], wqT)
            nc.scalar.dma_start(wk_sb[:], wkT)
            nc.scalar.dma_start(wv_sb[:], wvT)
            nc.scalar.dma_start(wo_sb[:], woT)
            bq_sb = const.tile([P, 2], F32, tag="bq", name="bq_sb")
            bk_sb = const.tile([P, 2], F32, tag="bk", name="bk_sb")
            nc.scalar.dma_start(bq_sb[:], bq_v)
            nc.scalar.dma_start(bk_sb[:], bk_v)
            bv_sb = const.tile([1, DG], BF16, tag="bv", name="bv_sb")
            nc.scalar.dma_start(bv_sb[:], bv_d[None, :])
            ones_sb = const.tile([1, P], BF16, tag="ones", name="ones_sb")
            nc.scalar.dma_start(ones_sb[:], ones_d[None, :])
            b_sch = const.tile([P, 1], F32, tag="bsch", name="b_sch")
            nc.vector.memset(b_sch[:], SCH_B)

            kt_sb = const.tile([P, 2, N], BF16, tag="kt", name="kt_sb")
            v_sb = const.tile([P, NKT, HG, DK + 1], BF16, tag="v", name="v_sb")
            # ones column for the softmax-denominator rows: engine memset (a
            # strided broadcast DMA here generates thousands of 2-byte
            # packets and lands ~100us in, stalling every AV weight load)
            nc.vector.memset(
                v_sb[:, :, :, DK].rearrange("p a b -> p (a b)"), 1.0)

            # single-bank [128, 512] psum tiles; two tags so consecutive
            # users double-buffer and the scalar/DVE exp readers never share
            # a tile (sharing makes Tile proxy one engine's PE-dependency
            # through the other's semaphore, serializing them)
            ping = [0]

            def proj_ps(name):
                tag = "sts" if ping[0] == 0 else "std"
                ping[0] ^= 1
                return pp.tile([P, 512], F32, tag=tag, name=name)

            def kt_proj(c4, xk):
                for j in range(2):
                    ps = proj_ps(f"kps_{c4}_{j}")
                    for k in range(KT):
                        nc.tensor.matmul(
                            ps[:], wk_sb[:, k, j * P:(j + 1) * P], xk[:, k, :],
                            start=(k == 0), stop=(k == KT - 1))
                    nc.scalar.activation(
                        kt_sb[:, j, c4 * 512:(c4 + 1) * 512], ps[:], IDENT,
                        bias=bk_sb[:, j, None])

            def v_proj_grp(g2, xvs):
                vps = proj_ps(f"vps_{g2}")
                for q in range(2):
                    nt = g2 * 2 + q
                    xv = xvs[nt]
                    sl = vps[:, q * 256:(q + 1) * 256]
                    for k in range(KT):
                        nc.tensor.matmul(sl, xv[:, k, :], wv_sb[:, k, :],
                                         start=(k == 0), stop=False)
                    nc.tensor.matmul(sl, ones_sb[:], bv_sb[:],
                                     start=False, stop=True)
                    dst = v_sb[:, nt, :, 0:DK]
                    src = sl.rearrange("p (h d) -> p h d", h=HG)
                    if q % 2 == 0:
                        nc.scalar.activation(dst, src, COPY)
                    else:
                        nc.vector.tensor_copy(dst, src)

            def xq_load(c):
                ts = []
                for qh in range(2):
                    t = xin.tile([P, KT, 512], BF16, tag="xq", bufs=4,
                                 name=f"xq_{c}_{qh}")
                    nc.sync.dma_start(t[:], xqT[c * 2 + qh])
                    ts.append(t)
                return ts

            def qt_proj_half(qt, xq_ts, j, c):
                for qh in range(2):
                    ps = proj_ps(f"qps_{c}_{j}_{qh}")
                    for k in range(KT):
                        nc.tensor.matmul(
                            ps[:], wq_sb[:, k, j * P:(j + 1) * P],
                            xq_ts[qh][:, k, :],
                            start=(k == 0), stop=(k == KT - 1))
                    nc.scalar.activation(
                        qt[:, j, qh * 512:(qh + 1) * 512], ps[:], IDENT,
                        bias=bq_sb[:, j, None])

            def attention_pair(c, pair, qt, o_sb, deferred, inject=None):
                avs = [None, None]
                ats = {}

                def S_half(it, which):
                    kt, hp = it >> 1, it & 1
                    p0 = 64 * hp
                    sc = (it >> 1) & 1        # scalar's query half (kt parity)
                    if which == 0:
                        st_s = pp.tile([P, 512], F32, tag="sts",
                                       name=f"sts_{c}_{pair}_{it}")
                        at_s = atp.tile([P, 512], BF16, tag="ats",
                                        name=f"ats_{c}_{pair}_{it}")
                        nc.tensor.matmul(
                            st_s[:],
                            kt_sb[p0:p0 + 64, pair, kt * P:(kt + 1) * P],
                            qt[p0:p0 + 64, pair, sc * 512:(sc + 1) * 512],
                            start=True, stop=True)
                        nc.scalar.activation(at_s[:], st_s[:], EXP,
                                             scale=0.125)
                        ats[it] = [at_s[:], None, sc]
                    else:
                        qh = 1 - sc
                        st_d = pp.tile([P, 512], F32, tag="std",
                                       name=f"std_{c}_{pair}_{it}")
                        at_d = atp.tile([P, 512], I16, tag="atd",
                                        name=f"atd_{c}_{pair}_{it}")
                        nc.tensor.matmul(
                            st_d[:],
                            kt_sb[p0:p0 + 64, pair, kt * P:(kt + 1) * P],
                            qt[p0:p0 + 64, pair, qh * 512:(qh + 1) * 512],
                            start=True, stop=True)
                        nc.vector.scalar_tensor_tensor(
                            at_d[:], st_d[:], SCH_A,
                            b_sch[:, 0, None].to_broadcast((P, 512)),
                            MULT, ADD)
                        ats[it][1] = at_d[:].bitcast(BF16)

                def AV_half(it, qh):
                    kt, hp = it >> 1, it & 1
                    h = 2 * pair + hp
                    if avs[hp] is None:
                        avs[hp] = pp.tile([DK + 1, QC], F32, tag="av",
                                          name=f"avs_{c}_{pair}_{hp}")
                    at_s, at_d, sc = ats[it]
                    src_at = at_s if qh == sc else at_d
                    nc.tensor.matmul(
                        avs[hp][:, qh * 512:(qh + 1) * 512],
                        v_sb[:, kt, h, :], src_at,
                        start=(kt == 0), stop=(kt == NKT - 1))

                for it in range(34):
                    if it < 32:
                        S_half(it, 0)
                        S_half(it, 1)
                    if it in (1, 2) and deferred:
                        deferred.pop(0)()
                    if it >= 2:
                        AV_half(it - 2, 0)
                        AV_half(it - 2, 1)
                    if inject and it in inject:
                        inject[it]()
                while deferred:
                    deferred.pop(0)()

                # softmax denominators: row 64 of each avs -> 1/z broadcast
                rzb = smal.tile([P, QC], F32, tag="rzb", name=f"rzb_{c}_{pair}")
                newdef = []
                for hp in range(2):
                    z1 = smal.tile([1, QC], F32, tag="z",
                                   name=f"z_{c}_{pair}_{hp}")
                    rz1 = smal.tile([1, QC], F32, tag="rz",
                                    name=f"rz_{c}_{pair}_{hp}")
                    nc.scalar.activation(z1[:], avs[hp][DK:DK + 1, :], COPY)
                    nc.vector.reciprocal_approx_fast(rz1[:], z1[:])
                    zrow = rz_st[(c * 2 + pair) * 2 + hp, None, :]
                    nc.gpsimd.dma_start(zrow, rz1[:])
                    nc.gpsimd.dma_start(
                        rzb[64 * hp:64 * hp + 64, :],
                        zrow.to_broadcast((64, QC)))

                    def mk(hp=hp):
                        def go():
                            nc.vector.tensor_tensor(
                                o_sb[64 * hp:64 * hp + 64, pair, :],
                                avs[hp][0:DK, :],
                                rzb[64 * hp:64 * hp + 64, :], MULT)
                        return go
                    newdef.append(mk())
                return newdef

            def out_proj(c, o_sb, ms):
                for m in ms:
                    y = smal.tile([P, QC], F32, tag="y", bufs=4,
                                  name=f"y_{c}_{m}")
                    for qh in range(2):
                        yps = proj_ps(f"yps_{c}_{m}_{qh}")
                        for j in range(2):
                            nc.tensor.matmul(
                                yps[:],
                                wo_sb[:, j, m * P:(m + 1) * P],
                                o_sb[:, j, qh * 512:(qh + 1) * 512],
                                start=(j == 0), stop=(j == 1))
                        dst = y[:, qh * 512:(qh + 1) * 512]
                        if (m + qh) % 2 == 0:
                            nc.scalar.activation(dst, yps[:], COPY)
                        else:
                            nc.vector.tensor_copy(dst, yps[:])
                    eng = nc.sync if m % 2 == 0 else nc.scalar
                    eng.dma_start(
                        yT_d[m * P:(m + 1) * P, c * QC:(c + 1) * QC], y[:])

            # ---- prefetch all inputs up front ----
            xq0 = xq_load(0)
            xks = []
            for c4 in range(0, 4):
                t = xin.tile([P, KT, 512], BF16, tag="xk", bufs=4,
                             name=f"xk_{c4}")
                nc.sync.dma_start(t[:], xkT[c4])
                xks.append(t)
            xvs = []
            for nt in range(NKT):
                t = xvp.tile([P, KT, P], BF16, tag="xv", bufs=16,
                             name=f"xv_{nt}")
                nc.gpsimd.dma_start(t[:], xvT[nt])
                xvs.append(t)

            # ---- minimal prologue: just enough for chunk0/pair0 to start.
            # The rest of the K/V/Q projections are injected between the
            # first attention pair's iterations so the input DMA streams
            # hide behind attention compute.
            qts = {}
            qts[0] = work.tile([P, 2, QC], BF16, tag="qt", name="qt_0")
            qt_proj_half(qts[0], xq0, 0, 0)
            kt_proj(0, xks[0])

            inject0 = {
                0:  lambda: v_proj_grp(0, xvs),
                4:  lambda: v_proj_grp(1, xvs),
                6:  lambda: kt_proj(1, xks[1]),
                8:  lambda: v_proj_grp(2, xvs),
                10: lambda: v_proj_grp(3, xvs),
                12: lambda: kt_proj(2, xks[2]),
                14: lambda: v_proj_grp(4, xvs),
                16: lambda: v_proj_grp(5, xvs),
                18: lambda: kt_proj(3, xks[3]),
                20: lambda: v_proj_grp(6, xvs),
                22: lambda: v_proj_grp(7, xvs),
                24: lambda: qt_proj_half(qts[0], xq0, 1, 0),
            }

            # ---- chunks ----
            # the previous chunk's pair1 o-multiplies carry across the chunk
            # seam as `deferred`, and its out-projection is injected between
            # the next chunk's pair0 iterations (PE filler while the softmax
            # denominator chain drains)
            carry = []            # deferred o-mults from previous pair1
            prev_out = None       # (c, o_sb) whose out-proj is still due
            for c in range(NCH):
                o_sb = work.tile([P, 2, QC], BF16, tag="o", name=f"o_{c}")
                if c + 1 < NCH:
                    xq_next = xq_load(c + 1)
                    qts[c + 1] = work.tile([P, 2, QC], BF16, tag="qt",
                                           name=f"qt_{c + 1}")
                if c == 0:
                    inj = inject0
                else:
                    pc, po = prev_out

                    def mk_op(ms, pc=pc, po=po):
                        return lambda: out_proj(pc, po, ms)
                    inj = {4: mk_op([0, 1]), 8: mk_op([2, 3]),
                           12: mk_op([4, 5]), 16: mk_op([6, 7])}
                    prev_out = None
                d0 = attention_pair(c, 0, qts[c], o_sb, carry, inject=inj)
                if c + 1 < NCH:
                    qt_proj_half(qts[c + 1], xq_next, 0, c + 1)
                d1 = attention_pair(c, 1, qts[c], o_sb, d0)
                if c + 1 < NCH:
                    qt_proj_half(qts[c + 1], xq_next, 1, c + 1)
                carry = d1
                prev_out = (c, o_sb)
            for go in carry:
                go()
            out_proj(prev_out[0], prev_out[1], range(0, 8))

    nc.compile()
    return nc


def get_program():
    global _PROGRAM
    if _PROGRAM is None:
        _PROGRAM = _build_program()
    return _PROGRAM


def _tile_xT(x, nchunk, width, dtype):
    # x [n, 1024] -> x^T tiled [nchunk, 128 p, 8 k, width]
    xt = np.ascontiguousarray(x.T)                      # [1024, n]
    return np.ascontiguousarray(
        xt.reshape(KT, P, nchunk, width).transpose(2, 1, 0, 3)).astype(dtype)


def _tile_w(w_rows, dtype):
    # w_rows [256, 1024] (= W[g-slice]) -> W^T tiled [128 p, 8 k, 256]
    return np.ascontiguousarray(
        w_rows.T.reshape(KT, P, DG).transpose(1, 0, 2)).astype(dtype)


def make_in_maps(queries, keys, values, Wq, bq, Wk, bk, Wv, bv, Wo, bo):
    """Build per-core input dicts. Core c handles batch c//4, head group c%4."""
    f32 = np.float32
    xT = {}
    for ib in range(B):
        xT[ib] = (
            _tile_xT(np.asarray(queries[ib], f32), 4, 512, NP_BF16),
            _tile_xT(np.asarray(keys[ib], f32), 4, 512, NP_BF16),
            _tile_xT(np.asarray(values[ib], f32), NKT, P, NP_BF16),
        )
    ones = np.ones((P,), NP_BF16)
    in_maps = []
    for core in range(8):
        ib, g = core // G, core % G
        sl = slice(g * DG, (g + 1) * DG)
        in_maps.append({
            "xqT": xT[ib][0], "xkT": xT[ib][1], "xvT": xT[ib][2],
            "wqT": _tile_w(Wq[sl, :], NP_BF16),
            "wkT": _tile_w(Wk[sl, :], NP_BF16),
            "wvT": _tile_w(Wv[sl, :], NP_BF16),
            "woT": np.ascontiguousarray(
                Wo[:, sl].T.reshape(2, P, D_MODEL).transpose(1, 0, 2)
            ).astype(NP_BF16),
            "bq_s": np.ascontiguousarray(bq[sl]).astype(f32),
            "bk_s": np.ascontiguousarray(bk[sl]).astype(f32),
            "bv_s": np.ascontiguousarray(bv[sl]).astype(NP_BF16),
            "ones_c": ones,
        })
    return in_maps


def gather_output(results, bo):
    out = np.zeros((B, N, D_MODEL), np.float32)
    for core in range(8):
        out[core // G] += np.asarray(results[core]["yT"], np.float32).T
    out += bo[None, None, :].astype(np.float32)
    return out


def _run(inputs, trace=False, **spmd_kwargs):
    nc = get_program()
    in_maps = make_in_maps(**inputs)
    res = run_bass_kernel_spmd(nc, in_maps, core_ids=list(range(8)),
                               trace=trace, **spmd_kwargs)
    return gather_output(res.results, inputs["bo"]), res


def kernel(**inputs) -> np.ndarray:
    out, _ = _run(inputs, trace=False)
    return out
